# revision 1
# baseline (speedup 1.0000x reference)
"""Trainium2 Bass kernel for nn_DiscreteContinuousConv2d (sparse DISCO conv).

Math (see reference):
    xq   = x * quadrature_weights               (B, C, n_in)
    xk   = segment_sum(xq[psi_idx_in] * psi_vals, k*n_out + psi_idx_out)
    out  = einsum("knbc,ock->bon", xk, weight) + bias

Kernel reformulation (Y-form): fold the channel/kernel mixing BEFORE the
sparse contraction.  With
    U[i, k, b, oc] = sum_c x[b, c, i] * weight[oc, c, k]        (dense, on PE)
and val'[e] = psi_vals[e] * qw[psi_idx_in[e]], the output is a pure
gather/segment-sum over the sparse entries:
    out[b, oc, o] = sum_{e in bin o} val'[e] * U[idx_in[e], idx_k[e], b, oc] + bias

Distribution: output-sharded over the 8 cores (core r owns o in
[r*2048, (r+1)*2048)); entries are bucketed per core on the host; no
collectives.  Per core:
    1. PE builds the U table (fp16, 16384*9 rows x 64) -> DRAM.
    2. Per 128-bin o-tile: dma_gather fetches each entry's U row.  The DMA
       gather element is 256B = a PAIR of adjacent U rows (j'//2); which half
       an entry needs is handled by splitting its value into an (A, B) pair
       (val, 0) or (0, val) so the segment-sum matmul consumes the full
       256B element and the two 64-wide halves are added at the end.
       Because gather indices are int16, entries are grouped by idx_in>>12
       (4 groups) and the gather reads from a per-group base offset.
    3. DVE builds a one-hot matrix over o_local from a shipped iota row;
       PE matmuls (lhsT=one-hot, rhs=scaled pairs) accumulate the 128-bin
       segment sums in PSUM; +bias, *2^-14 unscale; DMA out.

Host-side work is limited to index/layout preprocessing of the sparse
pattern (bucket/sort/pad, fold quadrature weights into per-entry values)
and the final unshard.
"""

import numpy as np
from contextlib import ExitStack

import concourse.bass as bass
import concourse.mybir as mybir
import concourse.tile as tile
from concourse import bacc
from concourse.bass_utils import run_bass_kernel_spmd

P = 128
N_CORES = 8
B, C, OC, K = 2, 32, 32, 9
N_IN = 16384
N_OUT = 16384
O_PER_CORE = N_OUT // N_CORES          # 2048
O_TILES = O_PER_CORE // P              # 16
V64 = B * OC                           # 64 values per U row
KV = K * V64                           # 576
N_GRP = 4                              # idx_in >> 12 gather groups
I_GRP = N_IN // N_GRP                  # 4096
ROWS_GRP = I_GRP * K                   # 36864 U rows per group (18432 pairs)
SCALE = np.float32(2.0 ** 14)          # keeps fp16 contributions in normal range

F16 = mybir.dt.float16
F32 = mybir.dt.float32
I16 = mybir.dt.int16


# --------------------------------------------------------------------------
# host-side preprocessing: pure index/layout work on the sparse pattern
# --------------------------------------------------------------------------

def _host_prep(inputs):
    qw = np.asarray(inputs["quadrature_weights"], np.float32)
    vals = np.asarray(inputs["psi_vals"], np.float32)
    ik = np.asarray(inputs["psi_idx_k"]).astype(np.int64)
    io = np.asarray(inputs["psi_idx_out"]).astype(np.int64)
    ii = np.asarray(inputs["psi_idx_in"]).astype(np.int64)

    val2 = (vals * qw[ii] * SCALE).astype(np.float32)
    grp = ii >> 12                                   # gather group 0..3
    jloc = (ii & (I_GRP - 1)) * K + ik               # row within group < 36864
    jpair = (jloc >> 1).astype(np.int16)             # 256B pair index < 18432
    parity = (jloc & 1).astype(np.int64)

    # order: (o-tile) -> (group) -> arbitrary; G = padded entries per
    # (o-tile, group), uniform so the program is static across cores
    otile = io >> 7
    gkey = otile * N_GRP + grp
    counts = np.bincount(gkey, minlength=(N_OUT // P) * N_GRP)
    G = int(np.ceil(counts.max() / P) * P)
    t_e = N_GRP * G

    order = np.argsort(gkey, kind="stable")
    gk_s = gkey[order]
    io_s, jp_s, par_s, v_s = io[order], jpair[order], parity[order], val2[order]
    o_loc_s = (io_s & (P - 1)).astype(np.float16)
    bounds = np.searchsorted(gk_s, np.arange((N_OUT // P) * N_GRP + 1))

    nc_cols = t_e // P
    per_core = []
    for r in range(N_CORES):
        # per o-tile arrays in device entry-slot order (slot e -> [e%128, e//128])
        JW = np.zeros((O_TILES, N_GRP, G // 16, 16), np.int16)   # wrapped idx
        VAB = np.zeros((O_TILES, t_e, 2), np.float16)
        OL = np.zeros((O_TILES, t_e), np.float16)
        for t in range(O_TILES):
            for g in range(N_GRP):
                gk = (r * O_TILES + t) * N_GRP + g
                lo, hi = bounds[gk], bounds[gk + 1]
                n = hi - lo
                assert n <= G, (n, G)
                base = g * G
                JW[t, g, : (n + 15) // 16, :] = np.resize(
                    np.pad(jp_s[lo:hi], (0, (-n) % 16)), ((n + 15) // 16, 16))
                sl = slice(base, base + n)
                VAB[t, sl, 0] = np.where(par_s[lo:hi] == 0, v_s[lo:hi], 0)
                VAB[t, sl, 1] = np.where(par_s[lo:hi] == 1, v_s[lo:hi], 0)
                OL[t, sl] = o_loc_s[lo:hi]
        per_core.append((JW, VAB, OL))

    # device layouts
    jidx = np.zeros((N_CORES, P, O_TILES * N_GRP * (G // 16)), np.int16)
    vab = np.zeros((N_CORES, P, O_TILES * nc_cols * 2), np.float16)
    oloc = np.zeros((N_CORES, P, O_TILES * nc_cols), np.float16)
    for r in range(N_CORES):
        JW, VAB, OL = per_core[r]
        # idx: per (t,g) block of G/16 columns; wrapped rows replicated to
        # the 8 Q7 cores' partition groups
        jw = JW.reshape(O_TILES * N_GRP, G // 16, 16).transpose(2, 0, 1)
        jw = jw.reshape(16, -1)
        jidx[r] = np.tile(jw, (8, 1))
        # val pairs: entry slot e at [e%128, (e//128)*2 + ab]
        v = VAB.reshape(O_TILES, nc_cols, P, 2).transpose(2, 0, 1, 3)
        vab[r] = v.reshape(P, -1)
        o = OL.reshape(O_TILES, nc_cols, P).transpose(2, 0, 1)
        oloc[r] = o.reshape(P, -1)

    weight = np.asarray(inputs["weight"], np.float32)      # (OC, C, K)
    w16 = weight.transpose(1, 2, 0).reshape(C, K * OC).astype(np.float16)
    w16 = np.ascontiguousarray(np.concatenate([w16, w16], axis=0))  # (64, 288)

    bias = np.asarray(inputs["bias"], np.float32)
    bias_t = np.ascontiguousarray(
        np.broadcast_to(np.tile(bias, B)[None, :], (P, V64))).astype(np.float32)

    iota_t = np.ascontiguousarray(
        np.broadcast_to(np.arange(P, dtype=np.float16)[None, :], (P, P)))

    x = np.ascontiguousarray(np.asarray(inputs["x"], np.float32))
    common = dict(x=x, wt=w16, biasrow=bias_t, iotarow=iota_t)
    percore = [dict(jidx=np.ascontiguousarray(jidx[r]),
                    vab=np.ascontiguousarray(vab[r]),
                    oloc=np.ascontiguousarray(oloc[r])) for r in range(N_CORES)]
    return percore, common, G


# --------------------------------------------------------------------------
# device program
# --------------------------------------------------------------------------

def _build(G):
    t_e = N_GRP * G
    nc_cols = t_e // P
    gcols = G // 16
    nc = bacc.Bacc("TRN2", target_bir_lowering=False)

    x_d = nc.dram_tensor("x", [B, C, N_IN], F32, kind="ExternalInput")
    w_d = nc.dram_tensor("wt", [2 * C, K * OC], F16, kind="ExternalInput")
    bias_d = nc.dram_tensor("biasrow", [P, V64], F32, kind="ExternalInput")
    iota_d = nc.dram_tensor("iotarow", [P, P], F16, kind="ExternalInput")
    j_d = nc.dram_tensor("jidx", [P, O_TILES * N_GRP * gcols], I16,
                         kind="ExternalInput")
    v_d = nc.dram_tensor("vab", [P, O_TILES * nc_cols * 2], F16,
                         kind="ExternalInput")
    o_d = nc.dram_tensor("oloc", [P, O_TILES * nc_cols], F16,
                         kind="ExternalInput")
    u_d = nc.dram_tensor("U", [N_IN * K, V64], F16, kind="Internal")
    out_d = nc.dram_tensor("out", [O_PER_CORE, V64], F32, kind="ExternalOutput")

    with tile.TileContext(nc) as tc, ExitStack() as ctx:
        cpool = ctx.enter_context(tc.tile_pool(name="const", bufs=1))
        x16 = cpool.tile([2 * C, N_IN], F16)
        nc.gpsimd.dma_start(out=x16[:], in_=x_d[:].rearrange("b c n -> (b c) n"))
        w16 = cpool.tile([2 * C, K * OC], F16)
        nc.sync.dma_start(out=w16[:], in_=w_d[:])
        # staged through a DVE copy so downstream DVE ops read same-engine data
        bias_t0 = cpool.tile([P, V64], F32)
        nc.sync.dma_start(out=bias_t0[:], in_=bias_d[:])
        bias_t = cpool.tile([P, V64], F32)
        nc.vector.tensor_copy(out=bias_t[:], in_=bias_t0[:])
        iota_t0 = cpool.tile([P, P], F16)
        nc.sync.dma_start(out=iota_t0[:], in_=iota_d[:])
        iota_t = cpool.tile([P, P], F16)
        nc.vector.tensor_copy(out=iota_t[:], in_=iota_t0[:])

        # ---- U build: U[(i k), (b oc)] = sum_c x16[(b,c), i] w16[(b,c), (k,oc)]
        upool = ctx.enter_context(tc.tile_pool(name="usb", bufs=3))
        ypsum = ctx.enter_context(tc.tile_pool(name="ypsum", bufs=4, space="PSUM"))
        u_ch = u_d[:].rearrange("(n p k) v -> n p (k v)", p=P, k=K)
        for ch in range(N_IN // P):
            u_sb = upool.tile([P, KV], F16)
            u_v = u_sb[:].rearrange("p (k b2 oc) -> p k b2 oc", k=K, b2=B)
            for b in range(B):
                yp = ypsum.tile([P, K * OC], F32)
                nc.tensor.matmul(
                    out=yp[:],
                    lhsT=x16[b * C:(b + 1) * C, ch * P:(ch + 1) * P],
                    rhs=w16[b * C:(b + 1) * C, :],
                    start=True, stop=True)
                nc.vector.tensor_copy(
                    out=u_v[:, :, b, :],
                    in_=yp[:].rearrange("p (k oc) -> p k oc", k=K))
            nc.sync.dma_start(out=u_ch[ch], in_=u_sb[:])

        # every gather reads all of U: collapse the 128 write completions
        tc.strict_bb_all_engine_barrier()

        # ---- sparse gather + segment-sum, one 128-bin o-tile at a time
        ipool = ctx.enter_context(tc.tile_pool(name="idx", bufs=2))
        gpool = ctx.enter_context(tc.tile_pool(name="gath", bufs=2))
        opsum = ctx.enter_context(tc.tile_pool(name="opsum", bufs=2, space="PSUM"))
        rpool = ctx.enter_context(tc.tile_pool(name="res", bufs=2))
        u_pair = u_d[:].rearrange("(q two) v -> q (two v)", two=2)  # (73728, 128)
        for t in range(O_TILES):
            jt = ipool.tile([P, N_GRP * gcols], I16, tag="jt")
            nc.sync.dma_start(
                out=jt[:], in_=j_d[:, t * N_GRP * gcols:(t + 1) * N_GRP * gcols])
            vt0 = ipool.tile([P, nc_cols * 2], F16, tag="vt0")
            ot0 = ipool.tile([P, nc_cols], F16, tag="ot0")
            nc.sync.dma_start(
                out=vt0[:], in_=v_d[:, t * nc_cols * 2:(t + 1) * nc_cols * 2])
            nc.sync.dma_start(
                out=ot0[:], in_=o_d[:, t * nc_cols:(t + 1) * nc_cols])
            vt = ipool.tile([P, nc_cols, 2], F16, tag="vt")
            ot = ipool.tile([P, nc_cols], F16, tag="ot")
            nc.vector.tensor_copy(out=vt[:], in_=vt0[:].rearrange(
                "p (c ab) -> p c ab", ab=2))
            nc.vector.tensor_copy(out=ot[:], in_=ot0[:])

            g = gpool.tile([P, nc_cols, 2 * V64], F16, tag="g")
            gtc = G // P        # gather tiles per group
            for gr in range(N_GRP):
                nc.gpsimd.dma_gather(
                    g[:, gr * gtc:(gr + 1) * gtc, :],
                    u_pair[gr * (ROWS_GRP // 2):(gr + 1) * (ROWS_GRP // 2), :],
                    jt[:, gr * gcols:(gr + 1) * gcols],
                    G, G, 2 * V64, elem_step=2 * V64, single_packet=False)

            # scale in place (SBUF budget): g <- g * val_pair
            oh = gpool.tile([P, nc_cols, P], F16, tag="oh")
            ET_CHUNK = 8
            for cc in range(0, nc_cols, ET_CHUNK):
                w = min(ET_CHUNK, nc_cols - cc)
                es = slice(cc, cc + w)
                nc.vector.tensor_tensor(
                    out=g[:, es, :].rearrange("p c (ab v) -> p c ab v", ab=2),
                    in0=g[:, es, :].rearrange("p c (ab v) -> p c ab v", ab=2),
                    in1=vt[:, es, :, None].broadcast_to((P, w, 2, V64)),
                    op=mybir.AluOpType.mult)
                nc.vector.tensor_tensor(
                    out=oh[:, es, :],
                    in0=ot[:, es, None].broadcast_to((P, w, P)),
                    in1=iota_t[:, None, :].broadcast_to((P, w, P)),
                    op=mybir.AluOpType.is_equal)

            ps = opsum.tile([P, 2 * V64], F32)
            for et in range(nc_cols):
                nc.tensor.matmul(
                    out=ps[:], lhsT=oh[:, et, :], rhs=g[:, et, :],
                    start=(et == 0), stop=(et == nc_cols - 1))

            res = rpool.tile([P, V64], F32, tag="res")
            # fold the two 64-wide halves (one PSUM operand per op), unscale, +bias
            halfb = rpool.tile([P, V64], F32, tag="halfb")
            nc.vector.tensor_copy(out=halfb[:], in_=ps[:, V64:2 * V64])
            half = rpool.tile([P, V64], F32, tag="half")
            nc.vector.tensor_add(out=half[:], in0=ps[:, 0:V64], in1=halfb[:])
            nc.vector.scalar_tensor_tensor(
                out=res[:], in0=half[:], scalar=float(1.0 / SCALE), in1=bias_t[:],
                op0=mybir.AluOpType.mult, op1=mybir.AluOpType.add)
            res2 = rpool.tile([P, V64], F32, tag="res2")
            nc.vector.tensor_copy(out=res2[:], in_=res[:])
            nc.sync.dma_start(out=out_d[t * P:(t + 1) * P, :], in_=res2[:])

    nc.compile()
    return nc


_last_result = None


def kernel(**inputs) -> np.ndarray:
    global _last_result
    per_core, common, G = _host_prep(inputs)
    nc = _build(G)
    in_maps = [{**common, **pc} for pc in per_core]
    r = run_bass_kernel_spmd(nc, in_maps, core_ids=list(range(N_CORES)))
    _last_result = r
    out = np.concatenate([res["out"] for res in r.results], axis=0)  # (16384, 64)
    return np.ascontiguousarray(out.reshape(N_OUT, B, OC).transpose(1, 2, 0))


if __name__ == "__main__":
    rng = np.random.default_rng(0)
    NNZ = 1_500_000
    ins = dict(
        x=rng.standard_normal((B, C, N_IN)).astype(np.float32),
        quadrature_weights=(rng.uniform(0.5, 1.5, N_IN) / N_IN).astype(np.float32),
        psi_vals=rng.uniform(0, 1, NNZ).astype(np.float32),
        weight=(rng.standard_normal((OC, C, K)) / np.sqrt(C)).astype(np.float32),
        bias=np.zeros(OC, np.float32),
        psi_idx_k=rng.integers(0, K, NNZ).astype(np.int32),
        psi_idx_out=rng.integers(0, N_OUT, NNZ).astype(np.int32),
        psi_idx_in=rng.integers(0, N_IN, NNZ).astype(np.int32),
        n_out=N_OUT,
    )
    out = kernel(**ins)
    print("kernel out", out.shape, out.dtype, float(np.abs(out).mean()))



# revision 7
# speedup vs baseline: 1.0151x; 1.0151x over previous
"""Trainium2 Bass kernel for nn_DiscreteContinuousConv2d (sparse DISCO conv).

Math (see reference):
    xq   = x * quadrature_weights               (B, C, n_in)
    xk   = segment_sum(xq[psi_idx_in] * psi_vals, k*n_out + psi_idx_out)
    out  = einsum("knbc,ock->bon", xk, weight) + bias

Kernel reformulation (Y-form): fold the channel/kernel mixing BEFORE the
sparse contraction.  With
    U[i, k, b, oc] = sum_c x[b, c, i] * weight[oc, c, k]        (dense, on PE)
and val'[e] = psi_vals[e] * qw[psi_idx_in[e]], the output is a pure
gather/segment-sum over the sparse entries:
    out[b, oc, o] = sum_{e in bin o} val'[e] * U[idx_in[e], idx_k[e], b, oc] + bias

Distribution: output-sharded over the 8 cores (core r owns o in
[r*2048, (r+1)*2048)); entries are bucketed per core on the host; no
collectives.  Per core:
    1. PE builds the U table (fp16, 16384*9 rows x 64) -> DRAM, one
       block-diagonal-weight matmul chain per 128-row chunk (the (k,b,oc)
       expansion is baked into a 64x576 weight so the PSUM->SBUF copy is
       contiguous).
    2. Per 128-bin o-tile: dma_gather fetches each entry's U row.  The DMA
       gather element is 256B = a PAIR of adjacent U rows (j'//2); which half
       an entry needs is handled by splitting its value into an (A, B) pair
       (val, 0) or (0, val) so the segment-sum matmul consumes the full
       256B element and the two 64-wide halves are added at the end.
       Because gather indices are int16, entries are grouped by idx_in>>12
       (4 groups) and the gather reads from a per-group base offset.
       Per-(o-tile, group) descriptor counts are trimmed to the actual
       entry count (padded to 16 with negative = skipped indices); bin
       sizes are maxed across cores so the program stays SPMD-uniform.
    3. DVE builds a one-hot matrix over o_local from a shipped iota row;
       PE matmuls (lhsT=one-hot, rhs=scaled pairs) accumulate the 128-bin
       segment sums in PSUM; +bias, *2^-14 unscale; DMA out.

Host-side work is limited to index/layout preprocessing of the sparse
pattern (bucket/sort/pad, fold quadrature weights into per-entry values)
and the final unshard.
"""

import numpy as np
from contextlib import ExitStack

import concourse.bass as bass
import concourse.mybir as mybir
import concourse.tile as tile
from concourse import bacc
from concourse.bass_utils import run_bass_kernel_spmd

P = 128
N_CORES = 8
B, C, OC, K = 2, 32, 32, 9
N_IN = 16384
N_OUT = 16384
O_PER_CORE = N_OUT // N_CORES          # 2048
O_TILES = O_PER_CORE // P              # 16
V64 = B * OC                           # 64 values per U row
KV = K * V64                           # 576
N_GRP = 4                              # idx_in >> 12 gather groups
I_GRP = N_IN // N_GRP                  # 4096
ROWS_GRP = I_GRP * K                   # 36864 U rows per group (18432 pairs)
SCALE = np.float32(2.0 ** 14)          # keeps fp16 contributions in normal range

F16 = mybir.dt.float16
F32 = mybir.dt.float32
I16 = mybir.dt.int16


# --------------------------------------------------------------------------
# host-side preprocessing: pure index/layout work on the sparse pattern
# --------------------------------------------------------------------------

def _host_prep(inputs):
    qw = np.asarray(inputs["quadrature_weights"], np.float32)
    vals = np.asarray(inputs["psi_vals"], np.float32)
    ik = np.asarray(inputs["psi_idx_k"]).astype(np.int64)
    io = np.asarray(inputs["psi_idx_out"]).astype(np.int64)
    ii = np.asarray(inputs["psi_idx_in"]).astype(np.int64)

    val2 = (vals * qw[ii] * SCALE).astype(np.float32)
    grp = ii >> 12                                   # gather group 0..3
    jloc = (ii & (I_GRP - 1)) * K + ik               # row within group < 36864
    jpair = (jloc >> 1).astype(np.int16)             # 256B pair index < 18432
    parity = (jloc & 1).astype(np.int64)

    # order: (core, o-tile, group) -> arbitrary
    otile = io >> 7
    gkey = otile * N_GRP + grp
    n_bins = (N_OUT // P) * N_GRP
    order = np.argsort(gkey, kind="stable")
    gk_s = gkey[order]
    io_s, jp_s, par_s, v_s = io[order], jpair[order], parity[order], val2[order]
    o_loc_s = (io_s & (P - 1)).astype(np.float16)
    bounds = np.searchsorted(gk_s, np.arange(n_bins + 1))
    nn = (bounds[1:] - bounds[:-1]).reshape(N_CORES, O_TILES, N_GRP)

    # per-(t,g) descriptor count: max over cores, rounded up to 128
    # (num_idxs must stay a 128-multiple and indices valid — negative
    # "skip" indices wedge the device); pads point at pair 0 with val 0
    sgrid = np.maximum(P, ((nn.max(axis=0) + P - 1) // P) * P)    # (T, G)
    pad16 = sgrid.copy()
    cols16 = pad16 // 16
    scols = sgrid // P
    cols_t = scols.sum(axis=1)                 # per-tile slot columns
    jcols_t = cols16.sum(axis=1)               # per-tile idx columns
    tot_j = int(jcols_t.sum())
    tot_c = int(cols_t.sum())

    jidx = np.zeros((N_CORES, P, tot_j), np.int16)
    vab = np.zeros((N_CORES, P, tot_c * 2), np.float16)
    oloc = np.zeros((N_CORES, P, tot_c), np.float16)
    for r in range(N_CORES):
        joff = 0
        coff = 0
        for t in range(O_TILES):
            s_tot = int(cols_t[t]) * P
            VT = np.zeros((s_tot, 2), np.float16)
            OT = np.zeros((s_tot,), np.float16)
            sbase = 0
            for g in range(N_GRP):
                gk = (r * O_TILES + t) * N_GRP + g
                lo, hi = bounds[gk], bounds[gk + 1]
                n = hi - lo
                p16 = int(pad16[t, g])
                assert n <= p16
                jcol = np.zeros((p16,), np.int16)
                jcol[:n] = jp_s[lo:hi]
                # idx slot e -> [16-partition channel e%16, col e//16]
                jw = jidx[r, :, joff:joff + p16 // 16]
                jw[:] = np.tile(jcol.reshape(p16 // 16, 16).T, (8, 1))
                joff += p16 // 16
                sl = slice(sbase, sbase + n)
                VT[sl, 0] = np.where(par_s[lo:hi] == 0, v_s[lo:hi], 0)
                VT[sl, 1] = np.where(par_s[lo:hi] == 1, v_s[lo:hi], 0)
                OT[sl] = o_loc_s[lo:hi]
                sbase += int(sgrid[t, g])
            # slot e -> [e%128, e//128]
            nc_t = int(cols_t[t])
            vab[r, :, coff * 2:(coff + nc_t) * 2] = (
                VT.reshape(nc_t, P, 2).transpose(1, 0, 2).reshape(P, -1))
            oloc[r, :, coff:coff + nc_t] = (
                OT.reshape(nc_t, P).transpose(1, 0))
            coff += nc_t

    # block-diagonal (b) expansion of weight: (b,c) x (k,b2,oc)
    weight = np.asarray(inputs["weight"], np.float32)      # (OC, C, K)
    wt = weight.transpose(1, 2, 0)                         # (c, k, oc)
    w2 = np.zeros((B, C, K, B, OC), np.float32)
    for b in range(B):
        w2[b, :, :, b, :] = wt
    w2 = np.ascontiguousarray(w2.reshape(B * C, KV)).astype(np.float16)

    bias = np.asarray(inputs["bias"], np.float32)
    bias_t = np.ascontiguousarray(
        np.broadcast_to(np.tile(bias, B)[None, :], (P, V64))).astype(np.float32)

    iota_t = np.ascontiguousarray(
        np.broadcast_to(np.arange(P, dtype=np.float16)[None, :], (P, P)))

    x = np.ascontiguousarray(np.asarray(inputs["x"], np.float32))
    common = dict(x=x, w2=w2, biasrow=bias_t, iotarow=iota_t)
    percore = [dict(jidx=np.ascontiguousarray(jidx[r]),
                    vab=np.ascontiguousarray(vab[r]),
                    oloc=np.ascontiguousarray(oloc[r])) for r in range(N_CORES)]
    meta = dict(pad16=pad16, sgrid=sgrid, cols16=cols16, scols=scols,
                cols_t=cols_t, jcols_t=jcols_t, tot_j=tot_j, tot_c=tot_c)
    return percore, common, meta


# --------------------------------------------------------------------------
# device program
# --------------------------------------------------------------------------

def _build(meta):
    pad16, sgrid = meta["pad16"], meta["sgrid"]
    cols16, scols = meta["cols16"], meta["scols"]
    cols_t, jcols_t = meta["cols_t"], meta["jcols_t"]
    tot_j, tot_c = meta["tot_j"], meta["tot_c"]
    nc = bacc.Bacc("TRN2", target_bir_lowering=False)

    x_d = nc.dram_tensor("x", [B, C, N_IN], F32, kind="ExternalInput")
    w2_d = nc.dram_tensor("w2", [B * C, KV], F16, kind="ExternalInput")
    bias_d = nc.dram_tensor("biasrow", [P, V64], F32, kind="ExternalInput")
    iota_d = nc.dram_tensor("iotarow", [P, P], F16, kind="ExternalInput")
    j_d = nc.dram_tensor("jidx", [P, tot_j], I16, kind="ExternalInput")
    v_d = nc.dram_tensor("vab", [P, tot_c * 2], F16, kind="ExternalInput")
    o_d = nc.dram_tensor("oloc", [P, tot_c], F16, kind="ExternalInput")
    u_d = nc.dram_tensor("U", [N_IN * K, V64], F16, kind="Internal")
    out_d = nc.dram_tensor("out", [O_PER_CORE, V64], F32, kind="ExternalOutput")

    with tile.TileContext(nc) as tc, ExitStack() as ctx:
        cpool = ctx.enter_context(tc.tile_pool(name="const", bufs=1))
        x16 = cpool.tile([2 * C, N_IN], F16)
        nc.gpsimd.dma_start(out=x16[:], in_=x_d[:].rearrange("b c n -> (b c) n"))
        w2t = cpool.tile([2 * C, KV], F16)
        nc.sync.dma_start(out=w2t[:], in_=w2_d[:])
        # staged through a DVE copy so downstream DVE ops read same-engine data
        bias_t0 = cpool.tile([P, V64], F32)
        nc.sync.dma_start(out=bias_t0[:], in_=bias_d[:])
        bias_t = cpool.tile([P, V64], F32)
        nc.vector.tensor_copy(out=bias_t[:], in_=bias_t0[:])
        iota_t0 = cpool.tile([P, P], F16)
        nc.sync.dma_start(out=iota_t0[:], in_=iota_d[:])
        iota_t = cpool.tile([P, P], F16)
        nc.vector.tensor_copy(out=iota_t[:], in_=iota_t0[:])

        # ---- U build: U[(i k), (b oc)]; one chunk = two matmuls with the
        # block-diag weight (f = 512 + 64) and two contiguous PSUM->SBUF casts
        upool = ctx.enter_context(tc.tile_pool(name="usb", bufs=3))
        ypsum = ctx.enter_context(tc.tile_pool(name="ypsum", bufs=2, space="PSUM"))
        u_ch = u_d[:].rearrange("(n p k) v -> n p (k v)", p=P, k=K)
        for ch in range(N_IN // P):
            u_sb = upool.tile([P, KV], F16)
            yp1 = ypsum.tile([P, 512], F32)
            yp2 = ypsum.tile([P, KV - 512], F32)
            lhs = x16[:, ch * P:(ch + 1) * P]
            nc.tensor.matmul(out=yp1[:], lhsT=lhs, rhs=w2t[:, 0:512],
                             start=True, stop=True)
            nc.tensor.matmul(out=yp2[:], lhsT=lhs, rhs=w2t[:, 512:KV],
                             start=True, stop=True)
            nc.vector.tensor_copy(out=u_sb[:, 0:512], in_=yp1[:])
            nc.vector.tensor_copy(out=u_sb[:, 512:KV], in_=yp2[:])
            nc.sync.dma_start(out=u_ch[ch], in_=u_sb[:])

        # every gather reads all of U: collapse the 128 write completions
        tc.strict_bb_all_engine_barrier()

        # ---- sparse gather + segment-sum, one 128-bin o-tile at a time
        ipool = ctx.enter_context(tc.tile_pool(name="idx", bufs=2))
        gpool = ctx.enter_context(tc.tile_pool(name="gath", bufs=2))
        opsum = ctx.enter_context(tc.tile_pool(name="opsum", bufs=2, space="PSUM"))
        rpool = ctx.enter_context(tc.tile_pool(name="res", bufs=2))
        u_pair = u_d[:].rearrange("(q two) v -> q (two v)", two=2)  # (73728, 128)
        joff = 0
        coff = 0
        for t in range(O_TILES):
            nc_t = int(cols_t[t])
            jc_t = int(jcols_t[t])
            jt = ipool.tile([P, jc_t], I16, tag="jt")
            nc.sync.dma_start(out=jt[:], in_=j_d[:, joff:joff + jc_t])
            vt0 = ipool.tile([P, nc_t * 2], F16, tag="vt0")
            ot0 = ipool.tile([P, nc_t], F16, tag="ot0")
            nc.sync.dma_start(
                out=vt0[:], in_=v_d[:, coff * 2:(coff + nc_t) * 2])
            nc.sync.dma_start(out=ot0[:], in_=o_d[:, coff:coff + nc_t])
            vt = ipool.tile([P, nc_t, 2], F16, tag="vt")
            ot = ipool.tile([P, nc_t], F16, tag="ot")
            nc.vector.tensor_copy(out=vt[:], in_=vt0[:].rearrange(
                "p (c ab) -> p c ab", ab=2))
            nc.vector.tensor_copy(out=ot[:], in_=ot0[:])

            g = gpool.tile([P, nc_t, 2 * V64], F16, tag="g")
            jo = 0
            so = 0
            for gr in range(N_GRP):
                p16 = int(pad16[t, gr])
                sc = int(scols[t, gr])
                nc.gpsimd.dma_gather(
                    g[:, so:so + sc, :],
                    u_pair[gr * (ROWS_GRP // 2):(gr + 1) * (ROWS_GRP // 2), :],
                    jt[:, jo:jo + p16 // 16],
                    p16, p16, 2 * V64, elem_step=2 * V64, single_packet=False)
                jo += p16 // 16
                so += sc

            # scale in place (SBUF budget): g <- g * val_pair
            oh = gpool.tile([P, nc_t, P], F16, tag="oh")
            ET_CHUNK = 8
            for cc in range(0, nc_t, ET_CHUNK):
                w = min(ET_CHUNK, nc_t - cc)
                es = slice(cc, cc + w)
                nc.vector.tensor_tensor(
                    out=g[:, es, :].rearrange("p c (ab v) -> p c ab v", ab=2),
                    in0=g[:, es, :].rearrange("p c (ab v) -> p c ab v", ab=2),
                    in1=vt[:, es, :, None].broadcast_to((P, w, 2, V64)),
                    op=mybir.AluOpType.mult)
                nc.vector.tensor_tensor(
                    out=oh[:, es, :],
                    in0=ot[:, es, None].broadcast_to((P, w, P)),
                    in1=iota_t[:, None, :].broadcast_to((P, w, P)),
                    op=mybir.AluOpType.is_equal)

            ps = opsum.tile([P, 2 * V64], F32)
            for et in range(nc_t):
                nc.tensor.matmul(
                    out=ps[:], lhsT=oh[:, et, :], rhs=g[:, et, :],
                    start=(et == 0), stop=(et == nc_t - 1))

            res = rpool.tile([P, V64], F32, tag="res")
            # fold the two 64-wide halves (one PSUM operand per op), unscale, +bias
            halfb = rpool.tile([P, V64], F32, tag="halfb")
            nc.vector.tensor_copy(out=halfb[:], in_=ps[:, V64:2 * V64])
            half = rpool.tile([P, V64], F32, tag="half")
            nc.vector.tensor_add(out=half[:], in0=ps[:, 0:V64], in1=halfb[:])
            nc.vector.scalar_tensor_tensor(
                out=res[:], in0=half[:], scalar=float(1.0 / SCALE), in1=bias_t[:],
                op0=mybir.AluOpType.mult, op1=mybir.AluOpType.add)
            res2 = rpool.tile([P, V64], F32, tag="res2")
            nc.vector.tensor_copy(out=res2[:], in_=res[:])
            nc.sync.dma_start(out=out_d[t * P:(t + 1) * P, :], in_=res2[:])
            joff += jc_t
            coff += nc_t

    nc.compile()
    return nc


_last_result = None


def kernel(**inputs) -> np.ndarray:
    global _last_result
    per_core, common, meta = _host_prep(inputs)
    nc = _build(meta)
    in_maps = [{**common, **pc} for pc in per_core]
    r = run_bass_kernel_spmd(nc, in_maps, core_ids=list(range(N_CORES)))
    _last_result = r
    out = np.concatenate([res["out"] for res in r.results], axis=0)  # (16384, 64)
    return np.ascontiguousarray(out.reshape(N_OUT, B, OC).transpose(1, 2, 0))


if __name__ == "__main__":
    rng = np.random.default_rng(0)
    NNZ = 1_500_000
    ins = dict(
        x=rng.standard_normal((B, C, N_IN)).astype(np.float32),
        quadrature_weights=(rng.uniform(0.5, 1.5, N_IN) / N_IN).astype(np.float32),
        psi_vals=rng.uniform(0, 1, NNZ).astype(np.float32),
        weight=(rng.standard_normal((OC, C, K)) / np.sqrt(C)).astype(np.float32),
        bias=np.zeros(OC, np.float32),
        psi_idx_k=rng.integers(0, K, NNZ).astype(np.int32),
        psi_idx_out=rng.integers(0, N_OUT, NNZ).astype(np.int32),
        psi_idx_in=rng.integers(0, N_IN, NNZ).astype(np.int32),
        n_out=N_OUT,
    )
    out = kernel(**ins)
    print("kernel out", out.shape, out.dtype, float(np.abs(out).mean()))
